# revision 25
# baseline (speedup 1.0000x reference)
"""CapsuleLayer (dynamic routing) Trainium2 Bass kernel.

Reference computation (B=64, N=512, D=1024, NCAP=16, DCAP=64, R=3):
    u_hat = (x @ W).reshape(B, N, 16, 64).transpose(0, 2, 1, 3)
    b = 0
    for t in range(3):
        c = softmax(b, axis=capsule)                  # [B, 16, N]
        v = einsum('bij,bijk->bik', c, u_hat)         # [B, 16, 64]
        out = v / sqrt(sum(v^2, -1) + 1e-7)
        if t < 2: b = einsum('bik,bijk->bij', out, u_hat)

Key algebraic refactoring (never materialize the 68.7 GFLOP u_hat):
    v[b,i,:]  = (c[b,i,:] @ x[b]) @ W_blk[i]          # s = c@x  then  s@W blockdiag
    b[b,i,j]  = x[b,j,:] . (W_blk[i] @ out[b,i,:])    # wo = W_blk@out then x@wo
This cuts PE work ~14x. Sharding: data-parallel over batch, 8 batches/core,
W replicated. All big operands stored bf16 in SBUF, fp32 PSUM accumulation.

v3 structure (vs v2):
  - Engine queues execute in emission order, so every matmul pass emits ALL
    its MMs back-to-back and the dependent PSUM->SBUF copies + follow-up
    transposes are emitted AFTER the whole pass: the copies chase on
    DVE/ACT while later MMs still stream, and the transposes find their
    inputs ready — no PE-queue bubble between col-tiled groups. (This is
    what lets the 4-way col-tiled s/b matmul groups actually overlap on
    hardware; measured ~55us/rep steady-state vs ~132us for the
    serial-PE cost model.)
  - DMA order: dmask/rperm first (tiny, needed at t=0), x[0..5], all of W,
    then x[6..7]; the WT transposes are emitted between batch 5 and 6 so
    they fill the PE lull while the W+x tail is in flight.
  - xT copy-backs 7:1 DVE:ACT (DVE is ~2.4x faster per copy).
  - bT transposes and the bt4 staging tile are bf16 (b is in [-8,8]; bf16
    rounding of b perturbs c by <0.5%, well inside the error budget).
  - The deferred squash scale 1/||v|| is folded into the wo PSUM->SBUF copy
    (rows are (i,lb); rinv is permuted once by a tiny constant matmul), so
    the b-matmul output needs no per-quad scaling and the old per-quad
    rperm matmuls/copies disappear.
  - ||v||^2 accumulates per-nh-half (two ACT Squares) so the rsqrt chain
    starts half an SW earlier.
  - Final output scaled + DMA'd per 512-col half.

Layout conventions per core (BL = 8 local batches):
    x_sb  [128p, lb, jt, d]    p+128*jt = j           (bf16)
    xT_sb [128p, lb, dc, jf]   p+128*dc = d           (bf16, PE-transposed)
    W_sb  [128p, o, n]         W[o*128+p, n]          (bf16)
    WT_sb [128p, o, d]         W[d, o*128+p]          (bf16, PE-transposed)
    S/v/output rows ordered (lb, i): row = lb*16 + i
    O_expT / wo-psum rows/columns ordered (i, lb): col = i*8 + lb
The off-diagonal blocks of the dense S@W product are zeroed with a
host-provided block-diagonal mask; the masked [128, 1024] tile serves as
both the (unscaled) squash input and the zero-padded expanded-outputs
operand of the next routing iteration. The host extracts the diagonal
blocks at the end.
"""

import numpy as np
import ml_dtypes

import concourse.bass as bass
import concourse.bacc as bacc
import concourse.mybir as mybir
import concourse.tile as tile
from concourse.bass_utils import run_bass_kernel_spmd
from concourse.masks import make_identity

NCORES = 8
B, N, D = 64, 512, 1024
NCAP, DCAP, R = 16, 64, 3
EPS = 1e-7
BL = B // NCORES          # local batches per core = 8
JT = N // 128             # j tiles = 4
DC = D // 128             # d chunks = 8
KC = D // 128             # k' chunks (NCAP*DCAP = 1024) = 8
QUADS = BL // 4           # col-tiling groups of 4 batches = 2

F32 = mybir.dt.float32
BF16 = mybir.dt.bfloat16
I32 = mybir.dt.int32
AX = mybir.AxisListType
ALU = mybir.AluOpType
ACT = mybir.ActivationFunctionType

RSQRT_MAGIC = 0x5F3759DF


def build_kernel(debug=False, reps=1):
    nc = bacc.Bacc(
        "TRN2",
        target_bir_lowering=False,
        debug=False,
        enable_asserts=False,
        num_devices=NCORES,
    )

    x_dram = nc.dram_tensor("x", (BL, N, D), BF16, kind="ExternalInput")
    w_dram = nc.dram_tensor("W", (D, NCAP * DCAP), BF16, kind="ExternalInput")
    mask_dram = nc.dram_tensor("dmask", (128, NCAP * DCAP), BF16, kind="ExternalInput")
    # rpermT[k, m] = 1 iff k = lb*16+i and m = i*8+lb (i < 16): permutes the
    # rinv column vector from (lb,i) row order to the wo-psum's (i,lb) order.
    rperm_dram = nc.dram_tensor("rperm", (128, 128), F32, kind="ExternalInput")
    out_dram = nc.dram_tensor("out", (128, NCAP * DCAP), BF16, kind="ExternalOutput")

    with tile.TileContext(nc) as tc:
        with (
            tc.tile_pool(name="const", bufs=1) as cpool,
            tc.tile_pool(name="work", bufs=1) as wpool,
            tc.tile_pool(name="work2", bufs=2) as w2pool,
            tc.tile_pool(name="ptp", bufs=2, space="PSUM") as tp_pool,
            tc.tile_pool(name="ps4", bufs=2, space="PSUM") as s4_pool,
        ):
            # ---------------- persistent SBUF tensors ----------------
            ident_bf = cpool.tile([128, 128], BF16, tag="identb")
            make_identity(nc, ident_bf)

            x_sb = cpool.tile([128, BL, JT, D], BF16, tag="x")
            w_sb = cpool.tile([128, DC, NCAP * DCAP], BF16, tag="w")
            w_view = w_dram[:].rearrange("(o p) n -> p o n", p=128)

            dmask = cpool.tile([128, NCAP * DCAP], BF16, tag="dmask")
            nc.sync.dma_start(dmask[:], mask_dram[:])
            rpermT = cpool.tile([128, 128], F32, tag="rpermT")
            nc.sync.dma_start(rpermT[:], rperm_dram[:])

            # c (routing weights) padded to 32 cols per (lb, jt) so col-tiled
            # matmul strips write full 32-partition rows (zeros in the pad).
            c_all = cpool.tile([128, BL, JT, 32], BF16, tag="c")
            nc.gpsimd.memset(c_all[:], 0.0)
            # woT[d, (lb, i)] padded the same way: [.., lb, 0:16] real.
            wot = cpool.tile([128, DC, BL, 32], BF16, tag="wot")
            nc.gpsimd.memset(wot[:], 0.0)

            xt_sb = cpool.tile([128, BL, DC, N], BF16, tag="xt")
            s0t = cpool.tile([128, DC, BL], F32, tag="s0t")
            wt_sb = cpool.tile([128, KC, D], BF16, tag="wt")

            o_final = None
            rinv = None
            for _rep in range(reps):
              # DMA order: x[0..5], W, x[6..7] — W lands near the x tail so
              # the WT transposes (emitted between batch 5 and 6 below)
              # fill the PE lull there and the t=0 S@W is never W-blocked.
              for lb in range(6):
                nc.sync.dma_start(
                    x_sb[:, lb],
                    x_dram[lb].rearrange("(jt p) d -> p jt d", p=128),
                )
              for dc in range(DC):
                nc.sync.dma_start(w_sb[:, dc], w_view[:, dc])
              for lb in range(6, BL):
                nc.sync.dma_start(
                    x_sb[:, lb],
                    x_dram[lb].rearrange("(jt p) d -> p jt d", p=128),
                )

              # ------------- x transposes (xT[d, j] per batch) -------------
              # The PSUM->SBUF copy-backs also accumulate along j, yielding
              # s0T[d, lb] = sum_j x[lb, j, d] for free (t=0 has uniform c).
              # Engine queues execute in emission order, so the WT
              # transposes are emitted between batch 5 and batch 6: they
              # fill the PE lull while batch 6's DMA (queued after W) is
              # still in flight.
              def x_transposes(lb):
                for dc in range(DC):
                    pt = tp_pool.tile([128, 512], BF16, tag="tpb")
                    for jt in range(JT):
                        nc.tensor.transpose(
                            pt[:, jt * 128:(jt + 1) * 128],
                            x_sb[:, lb, jt, dc * 128:(dc + 1) * 128],
                            ident_bf,
                        )
                    if dc != 7:
                        nc.vector.tensor_scalar(
                            out=xt_sb[:, lb, dc], in0=pt[:],
                            scalar1=1.0, scalar2=None,
                            op0=ALU.mult, op1=ALU.add,
                            accum_out=s0t[:, dc, lb:lb + 1],
                        )
                    else:
                        nc.scalar.activation(
                            xt_sb[:, lb, dc], pt[:], ACT.Copy,
                            accum_out=s0t[:, dc, lb:lb + 1],
                        )

              for lb in range(6):
                x_transposes(lb)

              # ------------- WT via on-chip PE transposes of W -------------
              for dc in range(DC):
                for kh in range(2):
                    pw = tp_pool.tile([128, 4, 128], BF16, tag="tpb")
                    for kq in range(4):
                        kc = kh * 4 + kq
                        nc.tensor.transpose(
                            pw[:, kq],
                            w_sb[:, dc, kc * 128:(kc + 1) * 128],
                            ident_bf,
                        )
                    nc.any.tensor_copy(
                        out=wt_sb[:, kh * 4:(kh + 1) * 4,
                                  dc * 128:(dc + 1) * 128],
                        in_=pw[:],
                    )

              for lb in range(6, BL):
                x_transposes(lb)

              for t in range(R):
                # ---------------- s = c @ x ----------------
                st_all = w2pool.tile([128, DC, 128], BF16, tag="st")
                if t == 0:
                    # Uniform c: sT = s0T broadcast over the 16 capsule
                    # columns of each (lq, i) 64-block. Scale is irrelevant
                    # (normalization is deferred through rinv).
                    for dc in range(DC):
                        nc.any.tensor_copy(
                            out=st_all[:, dc].rearrange(
                                "p (q lq i) -> p q lq i", q=QUADS, lq=4),
                            in_=s0t[:, dc, :, None].rearrange(
                                "p (q lq) i -> p q lq i", q=QUADS
                            ).to_broadcast((128, QUADS, 4, NCAP)),
                        )
                else:
                    # col-tiled: 4 batches per PSUM tile. All 64 MMs are
                    # emitted back-to-back so the PE queue never bubbles;
                    # the PSUM->SBUF copies chase on DVE/ACT, and the sT
                    # transposes (emitted after ALL the MMs) find their
                    # inputs already copied — no PE stall.
                    pss, s4s = [], []
                    for q in range(QUADS):
                        ps = s4_pool.tile([128, D], F32, tag="s4")
                        s4 = w2pool.tile([128, D], BF16, tag="s4sb")
                        pss.append(ps)
                        s4s.append(s4)
                        for nh in range(2):
                            for jt in range(JT):
                                for lq in range(4):
                                    lb = q * 4 + lq
                                    nc.tensor.matmul(
                                        ps[32 * lq:32 * lq + 32,
                                           nh * 512:(nh + 1) * 512],
                                        c_all[:, lb, jt, :],
                                        x_sb[:, lb, jt, nh * 512:(nh + 1) * 512],
                                        start=(jt == 0), stop=(jt == JT - 1),
                                        tile_position=(0, 32 * lq),
                                        skip_group_check=True,
                                    )
                            if nh == 0:
                                nc.vector.tensor_copy(
                                    s4s[q][:, 0:512], ps[:, 0:512])
                            else:
                                nc.scalar.activation(
                                    s4s[q][:, 512:1024], ps[:, 512:1024],
                                    ACT.Copy)
                    for q in range(QUADS):
                        for dh in range(2):
                            # transpose into ST[d, (lb,i)]; 4 chunks per tile
                            pst = tp_pool.tile([128, 4, 128], BF16, tag="tpb")
                            for dq in range(4):
                                dc = dh * 4 + dq
                                nc.tensor.transpose(
                                    pst[:, dq],
                                    s4s[q][:, dc * 128:(dc + 1) * 128],
                                    ident_bf,
                                )
                            nc.any.tensor_copy(
                                out=st_all[:, dh * 4:(dh + 1) * 4,
                                           q * 64:(q + 1) * 64].rearrange(
                                    "p dc (lq i) -> p dc lq i", lq=4),
                                in_=pst[:].rearrange(
                                    "p dc (lq r) -> p dc lq r", lq=4)[..., 0:NCAP],
                            )

                # ------------- v' = S @ W (dense, diag-blocks used) ----------
                # nh-outer: each 512-col half of v' completes early so the
                # mask + Square (and the first oexpt transposes) overlap the
                # other half's matmuls.
                po = s4_pool.tile([128, NCAP * DCAP], F32, tag="s4")
                o_full = w2pool.tile([128, NCAP * DCAP], BF16, tag="ofull")
                ss_nh = wpool.tile([128, 2], F32, tag="ssnh")
                sq_tmp = wpool.tile([128, 512], BF16, tag="sqtmp")
                for nh in range(2):
                    for dc in range(DC):
                        nc.tensor.matmul(
                            po[:, nh * 512:(nh + 1) * 512],
                            st_all[:, dc],
                            w_sb[:, dc, nh * 512:(nh + 1) * 512],
                            start=(dc == 0), stop=(dc == DC - 1),
                        )
                    # mask off-diag (normalization deferred)
                    nc.vector.tensor_tensor(
                        o_full[:, nh * 512:(nh + 1) * 512],
                        po[:, nh * 512:(nh + 1) * 512],
                        dmask[:, nh * 512:(nh + 1) * 512], ALU.mult)
                    nc.scalar.activation(
                        sq_tmp[:], o_full[:, nh * 512:(nh + 1) * 512],
                        ACT.Square, accum_out=ss_nh[:, nh:nh + 1])
                # ||v||^2 per (lb,i) row, then rinv = rsqrt(ss) on DVE
                # (Quake bit-trick + 2 Newton steps; ss >> eps for this model
                # so the +eps is dropped). Runs parallel to the oexpt/wo path.
                ss = wpool.tile([128, 1], F32, tag="ss")
                nc.vector.tensor_tensor(
                    ss[:], ss_nh[:, 0:1], ss_nh[:, 1:2], ALU.add)
                rsq_i = wpool.tile([128, 1], I32, tag="rsqi")
                nc.vector.tensor_scalar(
                    out=rsq_i[:], in0=ss[:].bitcast(I32),
                    scalar1=1, scalar2=None, op0=ALU.arith_shift_right,
                )
                y0 = wpool.tile([128, 1], I32, tag="y0")
                nc.vector.tensor_scalar(
                    out=y0[:], in0=rsq_i[:],
                    scalar1=-1, scalar2=RSQRT_MAGIC, op0=ALU.mult, op1=ALU.add,
                )
                yf = y0[:].bitcast(F32)
                y2 = wpool.tile([128, 1], F32, tag="y2")
                hy2 = wpool.tile([128, 1], F32, tag="hy2")
                y1 = wpool.tile([128, 1], F32, tag="y1")
                nc.vector.tensor_tensor(y2[:], yf, yf, ALU.mult)
                nc.vector.scalar_tensor_tensor(
                    hy2[:], ss[:], -0.5, y2[:], ALU.mult, ALU.mult)
                nc.vector.scalar_tensor_tensor(
                    y1[:], hy2[:], 1.5, yf, ALU.add, ALU.mult)
                rinv = wpool.tile([128, 1], F32, tag="rinv")
                nc.vector.tensor_tensor(y2[:], y1[:], y1[:], ALU.mult)
                nc.vector.scalar_tensor_tensor(
                    hy2[:], ss[:], -0.5, y2[:], ALU.mult, ALU.mult)
                nc.vector.scalar_tensor_tensor(
                    rinv[:], hy2[:], 1.5, y1[:], ALU.add, ALU.mult)
                o_final = o_full

                # ---------------- b update (t < R-1) ----------------
                if t == R - 1:
                    continue

                # --- O_expT[k', (i,lb)] via PE transpose of masked outs
                # (bf16). 4 chunks share a PSUM tile; the single copy permutes
                # (lb,i) -> (i,lb) so each capsule pair is a contiguous
                # 16-col block for the wo matmul. ---
                oexpt = w2pool.tile([128, KC, 128], BF16, tag="oexpt")
                for kh in range(2):
                    pt = tp_pool.tile([128, 4, 128], BF16, tag="tpb")
                    for kq in range(4):
                        kc = kh * 4 + kq
                        nc.tensor.transpose(
                            pt[:, kq], o_full[:, kc * 128:(kc + 1) * 128],
                            ident_bf,
                        )
                    nc.any.tensor_copy(
                        out=oexpt[:, kh * 4:(kh + 1) * 4].rearrange(
                            "p kc (i l) -> p kc i l", l=BL),
                        in_=pt[:].rearrange("p kc (l i) -> p kc i l", l=BL),
                    )
                # --- wo[(i,lb), d] = oexpt^T @ WT (dense; zero blocks of
                # O_expT kill cross-capsule terms). The PSUM->SBUF copy
                # applies the deferred 1/||v|| row scale. woT via PE
                # transpose. nh-outer: each 512-col half of wo completes
                # early; its copy and woT transposes overlap the other
                # half's matmuls. ---
                pwf = s4_pool.tile([128, D], F32, tag="s4")
                wo_sb = w2pool.tile([128, D], BF16, tag="wosb")
                rinv_il = wpool.tile([128, 1], F32, tag="rinvil")
                for nh in range(2):
                    for kc in range(KC):
                        nc.tensor.matmul(
                            pwf[:, nh * 512:(nh + 1) * 512],
                            oexpt[:, kc, :],
                            wt_sb[:, kc, nh * 512:(nh + 1) * 512],
                            start=(kc == 0), stop=(kc == KC - 1),
                        )
                    if nh == 0:
                        # rinv permuted into (i,lb) order by a tiny constant
                        # matmul. Emitted AFTER the first wo half so the PE
                        # (in-order queue) reaches it ~1.7us after the
                        # oexpt transposes — the DVE rsqrt chain it waits on
                        # is done by then. The rinv_il copy precedes the wo
                        # copies in the DVE queue.
                        rinv_ip = tp_pool.tile([128, 1], F32, tag="tpf")
                        nc.tensor.matmul(rinv_ip[:], rpermT[:], rinv[:],
                                         start=True, stop=True)
                        nc.vector.tensor_copy(rinv_il[:], rinv_ip[:])
                    nc.vector.tensor_scalar(
                        out=wo_sb[:, nh * 512:(nh + 1) * 512],
                        in0=pwf[:, nh * 512:(nh + 1) * 512],
                        scalar1=rinv_il[:], scalar2=None, op0=ALU.mult,
                    )
                for dh in range(2):
                    pw = tp_pool.tile([128, 4, 128], BF16, tag="tpb")
                    for dq in range(4):
                        dc = dh * 4 + dq
                        nc.tensor.transpose(
                            pw[:, dq], wo_sb[:, dc * 128:(dc + 1) * 128],
                            ident_bf,
                        )
                    # pw cols are (i,lb); wot wants (lb,i)
                    nc.any.tensor_copy(
                        out=wot[:, dh * 4:(dh + 1) * 4, :, 0:NCAP],
                        in_=pw[:].rearrange("p dc (i l) -> p dc l i", l=BL),
                    )
                # --- bT[i, j] per batch = sum_d woT[d,i] xT[d,j],
                # col-tiled 4 batches per PSUM tile. wo is pre-scaled by
                # rinv, so softmax sees the correctly-normalized b. Both
                # quad's copy / bT transposes / softmax follow its MMs, so
                # quad 0's softmax runs while quad 1's MMs are still on the
                # PE and the next iteration's s-MMs for quad 0 unblock
                # early. ---
                for q in range(QUADS):
                    pb = tp_pool.tile([128, 512], F32, tag="tpf")
                    for dc in range(DC):
                        for lq in range(4):
                            lb = q * 4 + lq
                            nc.tensor.matmul(
                                pb[32 * lq:32 * lq + 32, :],
                                wot[:, dc, lb, :],
                                xt_sb[:, lb, dc],
                                start=(dc == 0), stop=(dc == DC - 1),
                                tile_position=(0, 32 * lq),
                                skip_group_check=True,
                            )
                    bt4 = wpool.tile([128, 512], BF16, tag=f"bt4_{q}")
                    nc.vector.tensor_copy(bt4[:], pb[:])
                    # transpose to b[j, i] layout; exp straight off the PSUM.
                    # b stays within [-8, 8] for this model, so skipping the
                    # softmax max-subtraction is safe, and bf16 rounding of b
                    # perturbs c well under the error budget.
                    ptb = tp_pool.tile([128, 4, 128], BF16, tag="tpb")
                    for jt in range(JT):
                        nc.tensor.transpose(
                            ptb[:, jt],
                            bt4[:, jt * 128:(jt + 1) * 128],
                            ident_bf,
                        )
                    eb = w2pool.tile([128, 4, JT, NCAP], F32, tag=f"eb{q}")
                    nc.scalar.activation(
                        eb[:].rearrange("p lq jt i -> p jt lq i"),
                        ptb[:].rearrange(
                            "p jt (lq r) -> p jt lq r", lq=4)[..., 0:NCAP],
                        ACT.Exp,
                    )
                    # --- per-quad softmax over the capsule axis, so this
                    # quad's c (and the next iteration's s-matmul on it) can
                    # proceed while the other quad's b-matmul still runs. ---
                    ebv = eb[:].rearrange("p lb jt i -> p (lb jt) i")
                    sumexp = wpool.tile([128, 4 * JT], F32, tag=f"sumexp{q}")
                    nc.vector.reduce_sum(sumexp[:], ebv, axis=AX.X)
                    rec = wpool.tile([128, 4 * JT], F32, tag=f"rec{q}")
                    nc.vector.reciprocal(rec[:], sumexp[:])
                    nc.vector.tensor_tensor(
                        c_all[:, q * 4:(q + 1) * 4, :, 0:NCAP].rearrange(
                            "p lb jt i -> p (lb jt) i"),
                        ebv,
                        rec[:, :, None].to_broadcast((128, 4 * JT, NCAP)),
                        ALU.mult,
                    )

            # ---------------- write result (apply deferred rinv) ----------
            out_sb = wpool.tile([128, NCAP * DCAP], BF16, tag="osb")
            for nh in range(2):
                nc.vector.tensor_scalar(
                    out=out_sb[:, nh * 512:(nh + 1) * 512],
                    in0=o_final[:, nh * 512:(nh + 1) * 512],
                    scalar1=rinv[:], scalar2=None, op0=ALU.mult,
                )
                nc.sync.dma_start(out_dram[:, nh * 512:(nh + 1) * 512],
                                  out_sb[:, nh * 512:(nh + 1) * 512])

    nc.compile()
    return nc


_NC_CACHE = {}


def _get_nc(debug=False):
    key = bool(debug)
    if key not in _NC_CACHE:
        _NC_CACHE[key] = build_kernel(debug=key)
    return _NC_CACHE[key]


def block_diag_mask():
    """dmask[lb*NCAP+i, n] = 1.0 iff n // DCAP == i (capsule i's block)."""
    m = np.zeros((128, NCAP * DCAP), dtype=np.float32)
    for lb in range(BL):
        for i in range(NCAP):
            m[lb * NCAP + i, i * DCAP:(i + 1) * DCAP] = 1.0
    return m


def rinv_perm():
    """rpermT[k, m] = 1 iff k = lb*16+i and m = i*8+lb (i < 16)."""
    p = np.zeros((128, 128), dtype=np.float32)
    for lb in range(BL):
        for i in range(NCAP):
            p[lb * NCAP + i, i * BL + lb] = 1.0
    return p


def make_in_maps(x, W):
    """Host-side prep: shard x over batch, cast bf16, replicate W."""
    assert x.shape == (B, N, D) and W.shape[-2:] == (D, NCAP * DCAP)
    w2 = np.ascontiguousarray(W.reshape(D, NCAP * DCAP)).astype(ml_dtypes.bfloat16)
    dm = block_diag_mask().astype(ml_dtypes.bfloat16)
    rp = rinv_perm()
    xb = x.astype(ml_dtypes.bfloat16)
    in_maps = []
    for c in range(NCORES):
        in_maps.append({
            "x": np.ascontiguousarray(xb[c * BL:(c + 1) * BL]),
            "W": w2,
            "dmask": dm,
            "rperm": rp,
        })
    return in_maps


def extract_out(core_out):
    """[128, 1024] masked tile -> [BL, NCAP, DCAP] (row lb*NCAP+i, block i)."""
    co = np.asarray(core_out, dtype=np.float32)
    r = np.empty((BL, NCAP, DCAP), dtype=np.float32)
    for i in range(NCAP):
        r[:, i, :] = co[i::NCAP, i * DCAP:(i + 1) * DCAP]
    return r


def kernel(x, W):
    nc = _get_nc(debug=False)
    in_maps = make_in_maps(np.asarray(x), np.asarray(W))
    res = run_bass_kernel_spmd(nc, in_maps, list(range(NCORES)))
    out = np.concatenate([extract_out(r["out"]) for r in res.results], axis=0)
    return out.astype(np.float32)


# revision 26
# speedup vs baseline: 1.3097x; 1.3097x over previous
"""CapsuleLayer (dynamic routing) Trainium2 Bass kernel.

Reference computation (B=64, N=512, D=1024, NCAP=16, DCAP=64, R=3):
    u_hat = (x @ W).reshape(B, N, 16, 64).transpose(0, 2, 1, 3)
    b = 0
    for t in range(3):
        c = softmax(b, axis=capsule)                  # [B, 16, N]
        v = einsum('bij,bijk->bik', c, u_hat)         # [B, 16, 64]
        out = v / sqrt(sum(v^2, -1) + 1e-7)
        if t < 2: b = einsum('bik,bijk->bij', out, u_hat)

Key algebraic refactoring (never materialize the 68.7 GFLOP u_hat):
    v[b,i,:]  = (c[b,i,:] @ x[b]) @ W_blk[i]          # s = c@x  then  s@W blockdiag
    b[b,i,j]  = x[b,j,:] . (W_blk[i] @ out[b,i,:])    # wo = W_blk@out then x@wo
This cuts PE work ~14x. Sharding: data-parallel over batch, 8 batches/core,
W replicated. All big operands stored bf16 in SBUF, fp32 PSUM accumulation.

v3 structure (vs v2):
  - Engine queues execute in emission order, so every matmul pass emits ALL
    its MMs back-to-back and the dependent PSUM->SBUF copies + follow-up
    transposes are emitted AFTER the whole pass: the copies chase on
    DVE/ACT while later MMs still stream, and the transposes find their
    inputs ready — no PE-queue bubble between col-tiled groups. (This is
    what lets the 4-way col-tiled s/b matmul groups actually overlap on
    hardware; measured ~55us/rep steady-state vs ~132us for the
    serial-PE cost model.)
  - DMA order: dmask/rperm first (tiny, needed at t=0), x[0..5], all of W,
    then x[6..7]; the WT transposes are emitted between batch 5 and 6 so
    they fill the PE lull while the W+x tail is in flight.
  - xT copy-backs 7:1 DVE:ACT (DVE is ~2.4x faster per copy).
  - bT transposes and the bt4 staging tile are bf16 (b is in [-8,8]; bf16
    rounding of b perturbs c by <0.5%, well inside the error budget).
  - The deferred squash scale 1/||v|| is folded into the wo PSUM->SBUF copy
    (rows are (i,lb); rinv is permuted once by a tiny constant matmul), so
    the b-matmul output needs no per-quad scaling and the old per-quad
    rperm matmuls/copies disappear.
  - ||v||^2 accumulates per-nh-half (two ACT Squares) so the rsqrt chain
    starts half an SW earlier.
  - Final output scaled + DMA'd per 512-col half.

Layout conventions per core (BL = 8 local batches):
    x_sb  [128p, lb, jt, d]    p+128*jt = j           (bf16)
    xT_sb [128p, lb, dc, jf]   p+128*dc = d           (bf16, PE-transposed)
    W_sb  [128p, o, n]         W[o*128+p, n]          (bf16)
    WT_sb [128p, o, d]         W[d, o*128+p]          (bf16, PE-transposed)
    S/v/output rows ordered (lb, i): row = lb*16 + i
    O_expT / wo-psum rows/columns ordered (i, lb): col = i*8 + lb
The off-diagonal blocks of the dense S@W product are zeroed with a
host-provided block-diagonal mask; the masked [128, 1024] tile serves as
both the (unscaled) squash input and the zero-padded expanded-outputs
operand of the next routing iteration. The host extracts the diagonal
blocks at the end.
"""

import numpy as np
import ml_dtypes

import concourse.bass as bass
import concourse.bacc as bacc
import concourse.mybir as mybir
import concourse.tile as tile
from concourse.bass_utils import run_bass_kernel_spmd
from concourse.masks import make_identity

NCORES = 8
B, N, D = 64, 512, 1024
NCAP, DCAP, R = 16, 64, 3
EPS = 1e-7
BL = B // NCORES          # local batches per core = 8
JT = N // 128             # j tiles = 4
DC = D // 128             # d chunks = 8
KC = D // 128             # k' chunks (NCAP*DCAP = 1024) = 8
QUADS = BL // 4           # col-tiling groups of 4 batches = 2

F32 = mybir.dt.float32
BF16 = mybir.dt.bfloat16
I32 = mybir.dt.int32
AX = mybir.AxisListType
ALU = mybir.AluOpType
ACT = mybir.ActivationFunctionType

RSQRT_MAGIC = 0x5F3759DF


def build_kernel(debug=False, reps=1):
    nc = bacc.Bacc(
        "TRN2",
        target_bir_lowering=False,
        debug=False,
        enable_asserts=False,
        num_devices=NCORES,
    )

    x_dram = nc.dram_tensor("x", (BL, N, D), BF16, kind="ExternalInput")
    w_dram = nc.dram_tensor("W", (D, NCAP * DCAP), BF16, kind="ExternalInput")
    mask_dram = nc.dram_tensor("dmask", (128, NCAP * DCAP), BF16, kind="ExternalInput")
    # rpermT[k, m] = 1 iff k = lb*16+i and m = i*8+lb (i < 16): permutes the
    # rinv column vector from (lb,i) row order to the wo-psum's (i,lb) order.
    rperm_dram = nc.dram_tensor("rperm", (128, 128), F32, kind="ExternalInput")
    out_dram = nc.dram_tensor("out", (128, NCAP * DCAP), BF16, kind="ExternalOutput")

    with tile.TileContext(nc) as tc:
        with (
            tc.tile_pool(name="const", bufs=1) as cpool,
            tc.tile_pool(name="work", bufs=1) as wpool,
            tc.tile_pool(name="work2", bufs=2) as w2pool,
            tc.tile_pool(name="ptp", bufs=2, space="PSUM") as tp_pool,
            tc.tile_pool(name="ps4", bufs=2, space="PSUM") as s4_pool,
        ):
            # ---------------- persistent SBUF tensors ----------------
            ident_bf = cpool.tile([128, 128], BF16, tag="identb")
            make_identity(nc, ident_bf)

            x_sb = cpool.tile([128, BL, JT, D], BF16, tag="x")
            w_sb = cpool.tile([128, DC, NCAP * DCAP], BF16, tag="w")
            w_view = w_dram[:].rearrange("(o p) n -> p o n", p=128)

            dmask = cpool.tile([128, NCAP * DCAP], BF16, tag="dmask")
            nc.sync.dma_start(dmask[:], mask_dram[:])
            rpermT = cpool.tile([128, 128], F32, tag="rpermT")
            nc.sync.dma_start(rpermT[:], rperm_dram[:])

            # c (routing weights) padded to 32 cols per (lb, jt) so col-tiled
            # matmul strips write full 32-partition rows (zeros in the pad).
            c_all = cpool.tile([128, BL, JT, 32], BF16, tag="c")
            nc.gpsimd.memset(c_all[:], 0.0)
            # woT[d, (lb, i)] padded the same way: [.., lb, 0:16] real.
            wot = cpool.tile([128, DC, BL, 32], BF16, tag="wot")
            nc.gpsimd.memset(wot[:], 0.0)

            xt_sb = cpool.tile([128, BL, DC, N], BF16, tag="xt")
            s0t = cpool.tile([128, DC, BL], F32, tag="s0t")
            wt_sb = cpool.tile([128, KC, D], BF16, tag="wt")

            o_final = None
            rinv = None
            for _rep in range(reps):
              # DMA order: x[0..5], W, x[6..7] — W lands near the x tail so
              # the WT transposes (emitted between batch 5 and 6 below)
              # fill the PE lull there and the t=0 S@W is never W-blocked.
              for lb in range(6):
                nc.sync.dma_start(
                    x_sb[:, lb],
                    x_dram[lb].rearrange("(jt p) d -> p jt d", p=128),
                )
              for dc in range(DC):
                nc.sync.dma_start(w_sb[:, dc], w_view[:, dc])
              for lb in range(6, BL):
                nc.sync.dma_start(
                    x_sb[:, lb],
                    x_dram[lb].rearrange("(jt p) d -> p jt d", p=128),
                )

              # ------------- x transposes (xT[d, j] per batch) -------------
              # The PSUM->SBUF copy-backs also accumulate along j, yielding
              # s0T[d, lb] = sum_j x[lb, j, d] for free (t=0 has uniform c).
              # Engine queues execute in emission order, so the WT
              # transposes are emitted between batch 5 and batch 6: they
              # fill the PE lull while batch 6's DMA (queued after W) is
              # still in flight.
              def x_transposes(lb):
                for dc in range(DC):
                    pt = tp_pool.tile([128, 512], BF16, tag="tpb")
                    for jt in range(JT):
                        nc.tensor.transpose(
                            pt[:, jt * 128:(jt + 1) * 128],
                            x_sb[:, lb, jt, dc * 128:(dc + 1) * 128],
                            ident_bf,
                        )
                    if dc != 7:
                        nc.vector.tensor_scalar(
                            out=xt_sb[:, lb, dc], in0=pt[:],
                            scalar1=1.0, scalar2=None,
                            op0=ALU.mult, op1=ALU.add,
                            accum_out=s0t[:, dc, lb:lb + 1],
                        )
                    else:
                        nc.scalar.activation(
                            xt_sb[:, lb, dc], pt[:], ACT.Copy,
                            accum_out=s0t[:, dc, lb:lb + 1],
                        )

              for lb in range(6):
                x_transposes(lb)

              # ------------- WT via on-chip PE transposes of W -------------
              for dc in range(DC):
                for kh in range(2):
                    pw = tp_pool.tile([128, 4, 128], BF16, tag="tpb")
                    for kq in range(4):
                        kc = kh * 4 + kq
                        nc.tensor.transpose(
                            pw[:, kq],
                            w_sb[:, dc, kc * 128:(kc + 1) * 128],
                            ident_bf,
                        )
                    nc.any.tensor_copy(
                        out=wt_sb[:, kh * 4:(kh + 1) * 4,
                                  dc * 128:(dc + 1) * 128],
                        in_=pw[:],
                    )

              for lb in range(6, BL):
                x_transposes(lb)

              for t in range(R):
                # ---------------- s = c @ x ----------------
                st_all = w2pool.tile([128, DC, 128], BF16, tag="st")
                if t == 0:
                    # Uniform c: sT = s0T broadcast over the 16 capsule
                    # columns of each (lq, i) 64-block. Scale is irrelevant
                    # (normalization is deferred through rinv).
                    # SBUF->SBUF broadcast on the (otherwise idle) Pool
                    # engine so DVE stays free for the last batches' xT
                    # copy-backs, which gate s0t and hence this very copy.
                    for dc in range(DC):
                        nc.gpsimd.tensor_copy(
                            out=st_all[:, dc].rearrange(
                                "p (q lq i) -> p q lq i", q=QUADS, lq=4),
                            in_=s0t[:, dc, :, None].rearrange(
                                "p (q lq) i -> p q lq i", q=QUADS
                            ).to_broadcast((128, QUADS, 4, NCAP)),
                        )
                else:
                    # col-tiled: 4 batches per PSUM tile. All 64 MMs are
                    # emitted back-to-back so the PE queue never bubbles;
                    # the PSUM->SBUF copies chase on DVE/ACT, and the sT
                    # transposes (emitted after ALL the MMs) find their
                    # inputs already copied — no PE stall.
                    pss, s4s = [], []
                    for q in range(QUADS):
                        ps = s4_pool.tile([128, D], F32, tag="s4")
                        s4 = w2pool.tile([128, D], BF16, tag="s4sb")
                        pss.append(ps)
                        s4s.append(s4)
                        for nh in range(2):
                            for jt in range(JT):
                                for lq in range(4):
                                    lb = q * 4 + lq
                                    nc.tensor.matmul(
                                        ps[32 * lq:32 * lq + 32,
                                           nh * 512:(nh + 1) * 512],
                                        c_all[:, lb, jt, :],
                                        x_sb[:, lb, jt, nh * 512:(nh + 1) * 512],
                                        start=(jt == 0), stop=(jt == JT - 1),
                                        tile_position=(0, 32 * lq),
                                        skip_group_check=True,
                                    )
                            if nh == 0:
                                nc.vector.tensor_copy(
                                    s4s[q][:, 0:512], ps[:, 0:512])
                            else:
                                nc.scalar.activation(
                                    s4s[q][:, 512:1024], ps[:, 512:1024],
                                    ACT.Copy)
                    for q in range(QUADS):
                        for dh in range(2):
                            # transpose into ST[d, (lb,i)]; 4 chunks per tile
                            pst = tp_pool.tile([128, 4, 128], BF16, tag="tpb")
                            for dq in range(4):
                                dc = dh * 4 + dq
                                nc.tensor.transpose(
                                    pst[:, dq],
                                    s4s[q][:, dc * 128:(dc + 1) * 128],
                                    ident_bf,
                                )
                            nc.any.tensor_copy(
                                out=st_all[:, dh * 4:(dh + 1) * 4,
                                           q * 64:(q + 1) * 64].rearrange(
                                    "p dc (lq i) -> p dc lq i", lq=4),
                                in_=pst[:].rearrange(
                                    "p dc (lq r) -> p dc lq r", lq=4)[..., 0:NCAP],
                            )

                # ------------- v' = S @ W (dense, diag-blocks used) ----------
                # nh-outer: each 512-col half of v' completes early so the
                # mask + Square (and the first oexpt transposes) overlap the
                # other half's matmuls.
                po = s4_pool.tile([128, NCAP * DCAP], F32, tag="s4")
                o_full = w2pool.tile([128, NCAP * DCAP], BF16, tag="ofull")
                ss_nh = wpool.tile([128, 2], F32, tag="ssnh")
                sq_tmp = wpool.tile([128, 512], BF16, tag="sqtmp")
                for nh in range(2):
                    for dc in range(DC):
                        nc.tensor.matmul(
                            po[:, nh * 512:(nh + 1) * 512],
                            st_all[:, dc],
                            w_sb[:, dc, nh * 512:(nh + 1) * 512],
                            start=(dc == 0), stop=(dc == DC - 1),
                        )
                    # mask off-diag (normalization deferred)
                    nc.vector.tensor_tensor(
                        o_full[:, nh * 512:(nh + 1) * 512],
                        po[:, nh * 512:(nh + 1) * 512],
                        dmask[:, nh * 512:(nh + 1) * 512], ALU.mult)
                    nc.scalar.activation(
                        sq_tmp[:], o_full[:, nh * 512:(nh + 1) * 512],
                        ACT.Square, accum_out=ss_nh[:, nh:nh + 1])
                # ||v||^2 per (lb,i) row, then rinv = rsqrt(ss) on DVE
                # (Quake bit-trick + 2 Newton steps; ss >> eps for this model
                # so the +eps is dropped). Runs parallel to the oexpt/wo path.
                ss = wpool.tile([128, 1], F32, tag="ss")
                nc.vector.tensor_tensor(
                    ss[:], ss_nh[:, 0:1], ss_nh[:, 1:2], ALU.add)
                rsq_i = wpool.tile([128, 1], I32, tag="rsqi")
                nc.vector.tensor_scalar(
                    out=rsq_i[:], in0=ss[:].bitcast(I32),
                    scalar1=1, scalar2=None, op0=ALU.arith_shift_right,
                )
                y0 = wpool.tile([128, 1], I32, tag="y0")
                nc.vector.tensor_scalar(
                    out=y0[:], in0=rsq_i[:],
                    scalar1=-1, scalar2=RSQRT_MAGIC, op0=ALU.mult, op1=ALU.add,
                )
                yf = y0[:].bitcast(F32)
                y2 = wpool.tile([128, 1], F32, tag="y2")
                hy2 = wpool.tile([128, 1], F32, tag="hy2")
                y1 = wpool.tile([128, 1], F32, tag="y1")
                nc.vector.tensor_tensor(y2[:], yf, yf, ALU.mult)
                nc.vector.scalar_tensor_tensor(
                    hy2[:], ss[:], -0.5, y2[:], ALU.mult, ALU.mult)
                nc.vector.scalar_tensor_tensor(
                    y1[:], hy2[:], 1.5, yf, ALU.add, ALU.mult)
                rinv = wpool.tile([128, 1], F32, tag="rinv")
                nc.vector.tensor_tensor(y2[:], y1[:], y1[:], ALU.mult)
                nc.vector.scalar_tensor_tensor(
                    hy2[:], ss[:], -0.5, y2[:], ALU.mult, ALU.mult)
                nc.vector.scalar_tensor_tensor(
                    rinv[:], hy2[:], 1.5, y1[:], ALU.add, ALU.mult)
                o_final = o_full

                # ---------------- b update (t < R-1) ----------------
                if t == R - 1:
                    continue

                # --- O_expT[k', (i,lb)] via PE transpose of masked outs
                # (bf16). 4 chunks share a PSUM tile; the single copy permutes
                # (lb,i) -> (i,lb) so each capsule pair is a contiguous
                # 16-col block for the wo matmul. ---
                oexpt = w2pool.tile([128, KC, 128], BF16, tag="oexpt")
                for kh in range(2):
                    pt = tp_pool.tile([128, 4, 128], BF16, tag="tpb")
                    for kq in range(4):
                        kc = kh * 4 + kq
                        nc.tensor.transpose(
                            pt[:, kq], o_full[:, kc * 128:(kc + 1) * 128],
                            ident_bf,
                        )
                    nc.any.tensor_copy(
                        out=oexpt[:, kh * 4:(kh + 1) * 4].rearrange(
                            "p kc (i l) -> p kc i l", l=BL),
                        in_=pt[:].rearrange("p kc (l i) -> p kc i l", l=BL),
                    )
                # --- wo[(i,lb), d] = oexpt^T @ WT (dense; zero blocks of
                # O_expT kill cross-capsule terms). The PSUM->SBUF copy
                # applies the deferred 1/||v|| row scale. woT via PE
                # transpose. nh-outer: each 512-col half of wo completes
                # early; its copy and woT transposes overlap the other
                # half's matmuls. ---
                pwf = s4_pool.tile([128, D], F32, tag="s4")
                wo_sb = w2pool.tile([128, D], BF16, tag="wosb")
                rinv_il = wpool.tile([128, 1], F32, tag="rinvil")
                for nh in range(2):
                    for kc in range(KC):
                        nc.tensor.matmul(
                            pwf[:, nh * 512:(nh + 1) * 512],
                            oexpt[:, kc, :],
                            wt_sb[:, kc, nh * 512:(nh + 1) * 512],
                            start=(kc == 0), stop=(kc == KC - 1),
                        )
                    if nh == 0:
                        # rinv permuted into (i,lb) order by a tiny constant
                        # matmul. Emitted AFTER the first wo half so the PE
                        # (in-order queue) reaches it ~1.7us after the
                        # oexpt transposes — the DVE rsqrt chain it waits on
                        # is done by then. The rinv_il copy precedes the wo
                        # copies in the DVE queue.
                        rinv_ip = tp_pool.tile([128, 1], F32, tag="tpf")
                        nc.tensor.matmul(rinv_ip[:], rpermT[:], rinv[:],
                                         start=True, stop=True)
                        nc.vector.tensor_copy(rinv_il[:], rinv_ip[:])
                    nc.vector.tensor_scalar(
                        out=wo_sb[:, nh * 512:(nh + 1) * 512],
                        in0=pwf[:, nh * 512:(nh + 1) * 512],
                        scalar1=rinv_il[:], scalar2=None, op0=ALU.mult,
                    )
                for dh in range(2):
                    pw = tp_pool.tile([128, 4, 128], BF16, tag="tpb")
                    for dq in range(4):
                        dc = dh * 4 + dq
                        nc.tensor.transpose(
                            pw[:, dq], wo_sb[:, dc * 128:(dc + 1) * 128],
                            ident_bf,
                        )
                    # pw cols are (i,lb); wot wants (lb,i)
                    nc.any.tensor_copy(
                        out=wot[:, dh * 4:(dh + 1) * 4, :, 0:NCAP],
                        in_=pw[:].rearrange("p dc (i l) -> p dc l i", l=BL),
                    )
                # --- bT[i, j] per batch = sum_d woT[d,i] xT[d,j],
                # col-tiled 4 batches per PSUM tile. wo is pre-scaled by
                # rinv, so softmax sees the correctly-normalized b. Both
                # quad's copy / bT transposes / softmax follow its MMs, so
                # quad 0's softmax runs while quad 1's MMs are still on the
                # PE and the next iteration's s-MMs for quad 0 unblock
                # early. ---
                for q in range(QUADS):
                    pb = tp_pool.tile([128, 512], F32, tag="tpf")
                    for dc in range(DC):
                        for lq in range(4):
                            lb = q * 4 + lq
                            nc.tensor.matmul(
                                pb[32 * lq:32 * lq + 32, :],
                                wot[:, dc, lb, :],
                                xt_sb[:, lb, dc],
                                start=(dc == 0), stop=(dc == DC - 1),
                                tile_position=(0, 32 * lq),
                                skip_group_check=True,
                            )
                    bt4 = wpool.tile([128, 512], BF16, tag=f"bt4_{q}")
                    nc.vector.tensor_copy(bt4[:], pb[:])
                    # transpose to b[j, i] layout; exp straight off the PSUM.
                    # b stays within [-8, 8] for this model, so skipping the
                    # softmax max-subtraction is safe, and bf16 rounding of b
                    # perturbs c well under the error budget.
                    ptb = tp_pool.tile([128, 4, 128], BF16, tag="tpb")
                    for jt in range(JT):
                        nc.tensor.transpose(
                            ptb[:, jt],
                            bt4[:, jt * 128:(jt + 1) * 128],
                            ident_bf,
                        )
                    eb = w2pool.tile([128, 4, JT, NCAP], F32, tag=f"eb{q}")
                    nc.scalar.activation(
                        eb[:].rearrange("p lq jt i -> p jt lq i"),
                        ptb[:].rearrange(
                            "p jt (lq r) -> p jt lq r", lq=4)[..., 0:NCAP],
                        ACT.Exp,
                    )
                    # --- per-quad softmax over the capsule axis, so this
                    # quad's c (and the next iteration's s-matmul on it) can
                    # proceed while the other quad's b-matmul still runs. ---
                    ebv = eb[:].rearrange("p lb jt i -> p (lb jt) i")
                    sumexp = wpool.tile([128, 4 * JT], F32, tag=f"sumexp{q}")
                    nc.vector.reduce_sum(sumexp[:], ebv, axis=AX.X)
                    rec = wpool.tile([128, 4 * JT], F32, tag=f"rec{q}")
                    nc.vector.reciprocal(rec[:], sumexp[:])
                    nc.vector.tensor_tensor(
                        c_all[:, q * 4:(q + 1) * 4, :, 0:NCAP].rearrange(
                            "p lb jt i -> p (lb jt) i"),
                        ebv,
                        rec[:, :, None].to_broadcast((128, 4 * JT, NCAP)),
                        ALU.mult,
                    )

            # ---------------- write result (apply deferred rinv) ----------
            out_sb = wpool.tile([128, NCAP * DCAP], BF16, tag="osb")
            for nh in range(2):
                nc.vector.tensor_scalar(
                    out=out_sb[:, nh * 512:(nh + 1) * 512],
                    in0=o_final[:, nh * 512:(nh + 1) * 512],
                    scalar1=rinv[:], scalar2=None, op0=ALU.mult,
                )
                nc.sync.dma_start(out_dram[:, nh * 512:(nh + 1) * 512],
                                  out_sb[:, nh * 512:(nh + 1) * 512])

    nc.compile()
    return nc


_NC_CACHE = {}


def _get_nc(debug=False):
    key = bool(debug)
    if key not in _NC_CACHE:
        _NC_CACHE[key] = build_kernel(debug=key)
    return _NC_CACHE[key]


def block_diag_mask():
    """dmask[lb*NCAP+i, n] = 1.0 iff n // DCAP == i (capsule i's block)."""
    m = np.zeros((128, NCAP * DCAP), dtype=np.float32)
    for lb in range(BL):
        for i in range(NCAP):
            m[lb * NCAP + i, i * DCAP:(i + 1) * DCAP] = 1.0
    return m


def rinv_perm():
    """rpermT[k, m] = 1 iff k = lb*16+i and m = i*8+lb (i < 16)."""
    p = np.zeros((128, 128), dtype=np.float32)
    for lb in range(BL):
        for i in range(NCAP):
            p[lb * NCAP + i, i * BL + lb] = 1.0
    return p


def make_in_maps(x, W):
    """Host-side prep: shard x over batch, cast bf16, replicate W."""
    assert x.shape == (B, N, D) and W.shape[-2:] == (D, NCAP * DCAP)
    w2 = np.ascontiguousarray(W.reshape(D, NCAP * DCAP)).astype(ml_dtypes.bfloat16)
    dm = block_diag_mask().astype(ml_dtypes.bfloat16)
    rp = rinv_perm()
    xb = x.astype(ml_dtypes.bfloat16)
    in_maps = []
    for c in range(NCORES):
        in_maps.append({
            "x": np.ascontiguousarray(xb[c * BL:(c + 1) * BL]),
            "W": w2,
            "dmask": dm,
            "rperm": rp,
        })
    return in_maps


def extract_out(core_out):
    """[128, 1024] masked tile -> [BL, NCAP, DCAP] (row lb*NCAP+i, block i)."""
    co = np.asarray(core_out, dtype=np.float32)
    r = np.empty((BL, NCAP, DCAP), dtype=np.float32)
    for i in range(NCAP):
        r[:, i, :] = co[i::NCAP, i * DCAP:(i + 1) * DCAP]
    return r


def kernel(x, W):
    nc = _get_nc(debug=False)
    in_maps = make_in_maps(np.asarray(x), np.asarray(W))
    res = run_bass_kernel_spmd(nc, in_maps, list(range(NCORES)))
    out = np.concatenate([extract_out(r["out"]) for r in res.results], axis=0)
    return out.astype(np.float32)
